# revision 8
# baseline (speedup 1.0000x reference)
"""Multi-head attention (B=2, S=2048, EMB=1024, H=16) on 8 Trainium2 cores.

v4 -- hybrid batch x head sharding: core c owns batch c//4 and heads
{4g..4g+3} where g = c%4 (a 256-wide feature slice, two 128-wide head
PAIRS).  Each core projects ONLY its batch's 2048 tokens (halving HBM
reads vs the head-only split: 14 MB vs 26 MB) and writes a [2048, 1024]
bf16 partial output (4 MB); the host sums 4 partials per batch.

Per-core steady state is ACT(exp)-bound: 128 chunks x [128 keys,
2 heads x 512 q] exp at ~1.15us each = ~147us of scalar-engine time.
The PE work per chunk (score quad ~215ns + att pair ~430ns + ~2 windows
of projection fillers) fits inside the exp period, so the design goal is
a gapless ACT pipeline:

  - scores_T [k, q] per key-chunk via the 2x2 tile_position quad
    (2 heads x 2 key-halves, K=64 M=64 N=512, concurrent quadrants)
  - exp: one ACT call per chunk ([128, 1024] over a 2-bank PSUM tile,
    scale=1/8, no max subtraction; |scores|/8 < ~7 stays in range)
  - att: M=65 (64 V dims + ones column accumulating the softmax
    denominator), the two head MMs emitted ADJACENT so they pipeline at
    stream rate
  - V projected token-major with N=256 matmuls (both head pairs share
    one psum bank), 2x the efficiency of per-pair N=128 MMs
  - 10 garbage warm-up matmuls at t=0 lift the PE HAM clock gate
    (4/8 -> 8/8) before the first real projection; a dummy exp at t=0
    triggers the ~2.7us ACT table load under the DMA staging shadow
  - x-block DMAs issued in need order; filler units carry min_step
    gates so the in-order PE queue never blocks on an unlanded DMA
"""

from collections import deque
from contextlib import ExitStack

import numpy as np
import ml_dtypes

import concourse.bass as bass  # noqa: F401
import concourse.mybir as mybir
import concourse.tile as tile
from concourse import bacc
from concourse.bass_utils import run_bass_kernel_spmd

BF = mybir.dt.bfloat16
F32 = mybir.dt.float32

EMB = 1024
HEADS = 16
HD = EMB // HEADS          # 64
B, S = 2, 2048
N_CORES = 8
P = 128
NE = EMB // P              # 8 contraction chunks
FEATS = 256                # features per core (4 heads)
PAIRS = 2                  # head pairs per core
TOKS = S                   # tokens per core (its batch)
QB = 512                   # query block (psum bank width)
NQ = TOKS // QB            # 4 qblocks
NB = TOKS // QB            # 4 x-blocks per tensor
SB = TOKS // P             # 16 key chunks
NCH = PAIRS * NQ * SB      # 128 chunks
EXPF = mybir.ActivationFunctionType.Exp
SCALE = 1.0 / np.sqrt(HD)  # 0.125


def _build_nc():
    nc = bacc.Bacc(num_devices=N_CORES)
    dp = nc.declare_dram_parameter
    xq = dp("xq", [NB, P, NE, QB], BF, isOutput=False)
    xk = dp("xk", [NB, P, NE, QB], BF, isOutput=False)
    xv = dp("xv", [NB, P, NE, QB], BF, isOutput=False)
    WqT = dp("WqT", [P, NE, FEATS], BF, isOutput=False)
    WkT = dp("WkT", [P, NE, FEATS], BF, isOutput=False)
    WvT = dp("WvT", [P, NE, FEATS], BF, isOutput=False)
    WoT = dp("WoT", [P, PAIRS, EMB], BF, isOutput=False)
    bqp = dp("bqp", [P, PAIRS], F32, isOutput=False)
    bkp = dp("bkp", [P, PAIRS], F32, isOutput=False)
    bvp = dp("bvp", [P, PAIRS], F32, isOutput=False)
    out = dp("out", [TOKS, EMB], BF, isOutput=True)

    with tile.TileContext(nc) as tc, ExitStack() as ctx:
        wpool = ctx.enter_context(tc.tile_pool(name="wts", bufs=1))
        apool = ctx.enter_context(tc.tile_pool(name="acts", bufs=1))
        xpool = ctx.enter_context(tc.tile_pool(name="xin", bufs=12))
        ppool = ctx.enter_context(tc.tile_pool(name="probs", bufs=6))
        dpool = ctx.enter_context(tc.tile_pool(name="dn", bufs=2))
        opool = ctx.enter_context(tc.tile_pool(name="ob", bufs=2))
        genps = ctx.enter_context(tc.tile_pool(name="gen", bufs=2, space="PSUM"))
        scps = ctx.enter_context(tc.tile_pool(name="sc", bufs=2, space="PSUM"))
        attps = ctx.enter_context(tc.tile_pool(name="att", bufs=1, space="PSUM"))

        qhT = apool.tile([P, PAIRS, TOKS], BF, tag="qhT")
        khT = apool.tile([P, PAIRS, TOKS], BF, tag="khT")
        vhe = apool.tile([P, SB, PAIRS, 2, HD + 1], BF, tag="vhe")
        attT = apool.tile([P, PAIRS, TOKS], BF, tag="attT")

        Wq_sb = wpool.tile([P, NE, FEATS], BF, tag="WqT")
        Wk_sb = wpool.tile([P, NE, FEATS], BF, tag="WkT")
        Wv_sb = wpool.tile([P, NE, FEATS], BF, tag="WvT")
        Wo_sb = wpool.tile([P, PAIRS, EMB], BF, tag="WoT")
        bq_sb = wpool.tile([P, PAIRS], F32, tag="bqp")
        bk_sb = wpool.tile([P, PAIRS], F32, tag="bkp")
        bv_sb = wpool.tile([P, PAIRS], F32, tag="bvp")
        warm = wpool.tile([P, QB], BF, tag="warm")
        ones64 = wpool.tile([1, HD], F32, tag="ones64")

        # t=0 work with no DMA deps: ACT table load fires under the DMA
        # shadow; garbage matmuls lift the HAM clock gate (~3.4us of PE
        # activity) so the first real projections run at 2.4 GHz
        nc.vector.memset(warm[:], 0.0)
        nc.vector.memset(ones64[:], 1.0)
        dmy = dpool.tile([P, 8], BF, tag="dmy")
        nc.scalar.activation(dmy[:], warm[:, 0:8], EXPF, scale=SCALE)
        nc.vector.memset(vhe[:, :, :, :, HD], 1.0)

        # ---- input staging, one queue, in need order; the first k/q
        # blocks are split so the first projections pipeline behind the
        # DMA at kk-half granularity (Tile tracks subtile deps) ----
        xblocks = {}

        def stage_block(name, src, nb, split=False):
            xb = xpool.tile([P, NE, QB], BF, tag="xin", name=f"x_{name}{nb}")
            if split:
                nc.sync.dma_start(xb[:, 0:4, :], src[nb, :, 0:4, :])
                nc.sync.dma_start(xb[:, 4:8, :], src[nb, :, 4:8, :])
            else:
                nc.sync.dma_start(xb[:], src[nb, :, :, :])
            xblocks[(name, nb)] = xb

        nc.sync.dma_start(Wk_sb[:], WkT[:])
        nc.sync.dma_start(Wq_sb[:], WqT[:])
        stage_block("k", xk, 0, split=True)
        stage_block("q", xq, 0, split=True)
        nc.sync.dma_start(Wv_sb[:], WvT[:])
        stage_block("v", xv, 0)
        nc.sync.dma_start(bq_sb[:], bqp[:])
        nc.sync.dma_start(bk_sb[:], bkp[:])
        nc.sync.dma_start(bv_sb[:], bvp[:])
        stage_block("k", xk, 1)
        stage_block("v", xv, 1)
        stage_block("k", xk, 2)
        stage_block("v", xv, 2)
        stage_block("q", xq, 1)
        stage_block("k", xk, 3)
        stage_block("v", xv, 3)
        stage_block("q", xq, 2)
        stage_block("q", xq, 3)
        nc.sync.dma_start(Wo_sb[:], WoT[:])

        # garbage matmuls with no DMA deps: keep the PE busy from t=0 so
        # the HAM clock gate opens (~3.4us) and STAYS open until the first
        # real projection's inputs land (~10us)
        for _ in range(34):
            nc.tensor.matmul(
                genps.tile([P, QB], F32, tag="gen", name="warmps")[:],
                warm[:, 0:P], warm[:], start=True, stop=True,
            )

        # ---- emission helpers ----
        def qk_half(dst, W_sb, xname, b_sb, r, nb, half, box):
            """Half of one pair's 512-token q/k projection block."""
            xb = xblocks[(xname, nb)]
            if half == 0:
                box[0] = genps.tile(
                    [P, QB], F32, tag="gen", name=f"pj_{xname}_{r}_{nb}"
                )
            ps = box[0]
            for kk in range(half * 4, half * 4 + 4):
                nc.tensor.matmul(
                    ps[:],
                    W_sb[:, kk, r * P : (r + 1) * P],
                    xb[:, kk, :],
                    start=(kk == 0),
                    stop=(kk == NE - 1),
                )
            if half == 1:
                t0 = nb * QB
                nc.vector.tensor_scalar_add(
                    dst[:, r, t0 : t0 + QB], ps[:], b_sb[:, r : r + 1]
                )

        def v_half(c, half, box):
            """Token-major v projection of one 128-token chunk (N=256 MMs
            cover BOTH head pairs in a single psum bank)."""
            nb, ci = divmod(c, NB)
            xb = xblocks[("v", nb)]
            if half == 0:
                box[0] = genps.tile([P, QB], F32, tag="gen", name=f"pv_{c}")
            ps = box[0]
            for kk in range(half * 4, half * 4 + 4):
                nc.tensor.matmul(
                    ps[:, 0:FEATS],
                    xb[:, kk, ci * P : (ci + 1) * P],
                    Wv_sb[:, kk, :],
                    start=(kk == 0),
                    stop=(kk == NE - 1),
                )
            if half == 1:
                nc.vector.tensor_copy(
                    vhe[:, c, :, :, 0:HD],
                    ps[:, 0:FEATS].rearrange(
                        "p (r h d) -> p r h d", r=PAIRS, d=HD
                    ),
                )

        def v_chunk(c):
            for f in unit_halves(v_half, c):
                f()

        def unit_halves(fn, *args):
            box = [None]
            return [(lambda h=h: fn(*args, h, box)) for h in range(2)]

        prio: deque = deque()     # normalize tails + out-proj: no DMA deps
        fillers: deque = deque()  # (min_step, fn): gated on x-block arrival

        def fill(step, budget=1):
            done = 0
            while done < budget and prio:
                prio.popleft()()
                done += 1
            while done < budget and fillers and fillers[0][0] <= step:
                fillers.popleft()[1]()
                done += 1

        # chunk list: pair-major, then qblock, then key chunk
        chunks = [
            (r, qi, j) for r in range(PAIRS) for qi in range(NQ)
            for j in range(SB)
        ]

        def sc_chunk(r, qi, j):
            """Scores+exp for one key chunk: 2x2 matmul quad, one ACT."""
            q0 = qi * QB
            j0 = j * P
            sc = scps.tile([P, 2 * QB], F32, tag="sc", name=f"sc_{r}_{qi}_{j}")
            nc.tensor.matmul(
                sc[0:HD, 0:QB],
                khT[0:HD, r, j0 : j0 + HD],
                qhT[0:HD, r, q0 : q0 + QB],
                start=True, stop=True,
            )
            nc.tensor.matmul(
                sc[HD:P, 0:QB],
                khT[0:HD, r, j0 + HD : j0 + P],
                qhT[0:HD, r, q0 : q0 + QB],
                start=True, stop=True,
            )
            nc.tensor.matmul(
                sc[0:HD, QB:],
                khT[HD:P, r, j0 : j0 + HD],
                qhT[HD:P, r, q0 : q0 + QB],
                start=True, stop=True,
            )
            nc.tensor.matmul(
                sc[HD:P, QB:],
                khT[HD:P, r, j0 + HD : j0 + P],
                qhT[HD:P, r, q0 : q0 + QB],
                start=True, stop=True,
            )
            pr = ppool.tile([P, 2 * QB], BF, tag="pr", name=f"pr_{r}_{qi}_{j}")
            nc.scalar.activation(pr[:], sc[:], EXPF, scale=SCALE)
            return pr

        att_tiles = {}

        def get_att(r, qi):
            if (r, qi) not in att_tiles:
                att_tiles[(r, qi)] = attps.tile(
                    [HD + 1, 2, QB], F32, tag="att", name=f"att_{r}_{qi}"
                )
            return att_tiles[(r, qi)]

        def normalize_block(r, qi, attAB, last=False):
            # one copy drains the att psum fast (its slot is WAR-waited by
            # the next qblock's first att matmul in the in-order PE queue)
            q0 = qi * QB
            if last:
                au2 = attAB
            else:
                au2 = dpool.tile([HD + 1, 2, QB], F32, tag="au",
                                 name=f"au_{r}_{qi}")
                nc.vector.tensor_copy(au2[:, 0, :], attAB[:, 0, :])
                nc.vector.tensor_copy(au2[:, 1, :], attAB[:, 1, :])

            def tail(h):
                d0 = dpool.tile([1, QB], F32, tag=f"d0{h}",
                                name=f"d0_{r}_{qi}_{h}")
                nc.vector.tensor_copy(d0[:], au2[HD : HD + 1, h, :])
                r0 = dpool.tile([1, QB], F32, tag=f"r0{h}",
                                name=f"r0_{r}_{qi}_{h}")
                nc.vector.reciprocal_approx_fast(r0[:], d0[:])
                rb = dpool.tile([HD, QB], F32, tag=f"rb{h}",
                                name=f"rb_{r}_{qi}_{h}")
                nc.gpsimd.partition_broadcast(rb[:], r0[:])
                nc.vector.tensor_mul(
                    attT[h * HD : (h + 1) * HD, r, q0 : q0 + QB],
                    au2[0:HD, h, :],
                    rb[:],
                )

            if last:
                tail(0)
                tail(1)
            else:
                prio.append(lambda: tail(0))
                prio.append(lambda: tail(1))

        tail_mode = [False]

        def outproj_half(qi, t, half, box):
            t0 = qi * QB + t * P
            if half == 0:
                box[0] = opool.tile([P, EMB], BF, tag="ob",
                                    name=f"ob_{qi}_{t}")
            ob = box[0]
            ps = genps.tile([P, QB], F32, tag="gen", name=f"o_{qi}_{t}_{half}")
            for r in range(PAIRS):
                nc.tensor.matmul(
                    ps[:],
                    attT[:, r, t0 : t0 + P],
                    Wo_sb[:, r, half * QB : (half + 1) * QB],
                    start=(r == 0),
                    stop=(r == PAIRS - 1),
                )
            if tail_mode[0]:
                nc.scalar.copy(ob[:, half * QB : (half + 1) * QB], ps[:])
            else:
                nc.vector.tensor_copy(
                    ob[:, half * QB : (half + 1) * QB], ps[:]
                )
            if half == 1:
                nc.gpsimd.dma_start(out[t0 : t0 + P, :], ob[:])

        def push_outproj(qi):
            for t in range(QB // P):
                for f in unit_halves(outproj_half, qi, t):
                    prio.append(f)

        def att_half(i, h, pr):
            r, qi, j = chunks[i]
            attAB = get_att(r, qi)
            nc.tensor.matmul(
                attAB[:, h, :],
                vhe[:, j, r, h, :],
                pr[:, h * QB : (h + 1) * QB],
                start=(j == 0),
                stop=(j == SB - 1),
            )
            if h == 1 and j == SB - 1:
                last = i == NCH - 1
                normalize_block(r, qi, attAB, last=last)
                del att_tiles[(r, qi)]
                if r == PAIRS - 1:
                    if last:
                        for t in range(QB // P):
                            for f in unit_halves(outproj_half, qi, t):
                                prio.append(f)
                    else:
                        push_outproj(qi)

        # ---- head: minimal serial chain to the first exp ----
        for f in unit_halves(qk_half, khT, Wk_sb, "k", bk_sb, 0, 0):
            f()
        for f in unit_halves(qk_half, qhT, Wq_sb, "q", bq_sb, 0, 0):
            f()

        # filler queue: min_step approximates when the unit's x-block DMA
        # has landed (1 step ~ 1.15us of attention; step 0 ~ 10.5us abs)
        # (min_step, unit): min_step = min(DMA-arrival estimate, the step
        # the unit's output is first consumed - 1); sorted ascending so the
        # FIFO gate never blocks an eligible unit behind a later one
        pend = []
        arr_k = {1: 4, 2: 9, 3: 15}
        arr_v = {0: 0, 1: 6, 2: 11, 3: 17}
        arr_q = {1: 13, 2: 19, 3: 21}
        for nb in range(1, NB):
            for f in unit_halves(qk_half, khT, Wk_sb, "k", bk_sb, 0, nb):
                pend.append((min(arr_k[nb], 4 * nb - 1), f))
        for c in range(2, SB):
            for f in unit_halves(v_half, c):
                pend.append((min(arr_v[c // NB], c), f))
        for nb in range(1, NB):
            for f in unit_halves(qk_half, qhT, Wq_sb, "q", bq_sb, 0, nb):
                pend.append((min(arr_q[nb], 16 * nb - 2), f))
        for nb in range(NB):
            for f in unit_halves(qk_half, khT, Wk_sb, "k", bk_sb, 1, nb):
                pend.append((28 + 2 * nb, f))
        for nb in range(NB):
            for f in unit_halves(qk_half, qhT, Wq_sb, "q", bq_sb, 1, nb):
                pend.append((40 + 2 * nb, f))
        for ms, f in sorted(pend, key=lambda e: e[0]):
            fillers.append((ms, f))

        # ---- main pipeline: att emission lags the quad stream by 2
        # chunks so the att-psum drain copy has two exp periods to clear
        # its WAR before the next qblock's first att matmul reaches the
        # in-order PE queue head ----
        sc_pr = {}
        sc_pr[0] = sc_chunk(*chunks[0])
        v_chunk(0)
        v_chunk(1)
        for i in range(1, NCH):
            sc_pr[i] = sc_chunk(*chunks[i])
            if i >= 2:
                pr = sc_pr.pop(i - 2)
                att_half(i - 2, 0, pr)
                att_half(i - 2, 1, pr)
            if i % SB not in (SB - 1, 0):
                fill(i)
        for i in (NCH - 2, NCH - 1):
            pr = sc_pr.pop(i)
            if i == NCH - 1:
                tail_mode[0] = True
            att_half(i, 0, pr)
            att_half(i, 1, pr)
        for _ in range(6):
            nc.tensor.matmul(
                genps.tile([P, QB], F32, tag="gen", name="warmtail")[:],
                warm[:, 0:P], warm[:], start=True, stop=True,
            )
        while prio or fillers:
            fill(NCH, budget=4)

    nc.finalize()
    return nc


_NC_CACHE: list = [None]
_BO_CACHE: list = [None]


def _get_nc(*_args):
    if _NC_CACHE[0] is None:
        _NC_CACHE[0] = _build_nc()
    return _NC_CACHE[0]


def _feat_tiled(xT):
    """[EMB, n] -> [128, NE, n] contiguous (feature chunks on partitions)."""
    n = xT.shape[1]
    return np.ascontiguousarray(xT.reshape(NE, P, n).transpose(1, 0, 2))


def _stage(inputs):
    bf = ml_dtypes.bfloat16
    f32 = np.float32

    def arr(name):
        return np.asarray(inputs[name], f32)

    q, k, v = arr("q"), arr("k"), arr("v")
    Wq, Wk, Wv, Wo = arr("Wq"), arr("Wk"), arr("Wv"), arr("Wo")
    bq, bk, bv, bo = arr("bq"), arr("bk"), arr("bv"), arr("bo")
    _BO_CACHE[0] = bo if bool(np.any(bo)) else None

    def xt(x2d):  # [TOKS, EMB] -> [NB, 128, NE, QB] bf16 blocked
        xT = np.ascontiguousarray(x2d.T)  # [EMB, TOKS]
        blocks = [
            _feat_tiled(xT[:, i * QB : (i + 1) * QB]) for i in range(NB)
        ]
        return np.ascontiguousarray(np.stack(blocks)).astype(bf)

    xq_b = [xt(q[b]) for b in range(B)]
    xk_b = [xt(k[b]) for b in range(B)]
    xv_b = [xt(v[b]) for b in range(B)]

    in_maps = []
    for c in range(N_CORES):
        b = c // 4
        g = c % 4
        F = slice(g * FEATS, (g + 1) * FEATS)

        def bias_tiled(bvec):
            return np.ascontiguousarray(
                bvec[F].reshape(PAIRS, P).T
            ).astype(f32)

        m = {
            "xq": xq_b[b],
            "xk": xk_b[b],
            "xv": xv_b[b],
            "WqT": _feat_tiled(np.ascontiguousarray(Wq.T[:, F])).astype(bf),
            "WkT": _feat_tiled(np.ascontiguousarray(Wk.T[:, F])).astype(bf),
            "WvT": _feat_tiled(np.ascontiguousarray(Wv.T[:, F])).astype(bf),
            "WoT": np.ascontiguousarray(
                Wo.T[F, :].reshape(PAIRS, P, EMB).transpose(1, 0, 2)
            ).astype(bf),
            "bqp": bias_tiled(bq),
            "bkp": bias_tiled(bk),
            "bvp": bias_tiled(bv),
        }
        in_maps.append(m)
    return in_maps, True, True


def _assemble(results):
    full = np.empty((B, S, EMB), np.float32)
    for b in range(B):
        acc = results[4 * b]["out"].astype(np.float32)
        for g in range(1, 4):
            acc += results[4 * b + g]["out"].astype(np.float32)
        full[b] = acc
    if _BO_CACHE[0] is not None:
        full += _BO_CACHE[0]
    return full


def kernel(**inputs) -> np.ndarray:
    in_maps, _, _ = _stage(inputs)
    nc = _get_nc()
    res = run_bass_kernel_spmd(nc, in_maps, list(range(N_CORES)))
    return _assemble(res.results)
